# revision 3
# baseline (speedup 1.0000x reference)
"""Trainium2 Bass kernel for nn_MatrixFactorization (segment_reduce).

Decomposition (8 cores, SPMD, no collectives):
  - Dedup users of the batch -> unique users, sharded 8 ways (upc per core).
  - Host shards train_label[uniq].T per core in partition-major layout
    (contraction dim = items lands on SBUF partitions), zero-padded to
    157*128 rows.
  - Device streams the f32 label shard with SWDGE cast-DMA to bf16 (labels
    are exactly 0/1, so the cast is lossless) and accumulates
        P.T[66, upc] += T_aug_chunk.T @ L.T_chunk    (157 chunks of K=128)
    where T_aug = [item_table (row 20000 zeroed) | ones | 0] in bf16; column
    64 of P accumulates num_rel. PE transposes P.T back per 128-user block,
    DVE computes P[:, :64] * recip(P[:, 64]) -> uni_center rows.
  - Cluster centers: per chunk, DVE iota/is_equal builds onehot[128k, 256c];
    PE accumulates centers.T[66, 256] using the same T_aug chunks (the last
    chunk uses a variant with the real row 20000). counts ride the ones
    column; finalize = transpose, max(count,1), reciprocal, scale.
    Replicated on every core (hidden under the DMA-bound main loop).
  - user/pos/neg embeddings: GPSIMD indirect row gathers (256B rows),
    interleaved with the streaming loop.
  - pos/neg centers: exact fp32 onehot matmuls against the finalized
    centers (out as [64, nbpc]; host transposes back). Only one nonzero per
    onehot column, so this is an exact gather.
"""

import numpy as np
import ml_dtypes

import concourse.bass as bass
import concourse.mybir as mybir
import concourse.tile as tile

NUM_USERS = 10000
NUM_ITEMS = 20000
DIM = 64
CLUSTER = 256
BATCH = 8192
NCORES = 8

KCHUNKS = 157            # ceil(20001 / 128)
KPAD = KCHUNKS * 128     # 20096
MAUG = 66                # 64 dims + ones col + pad col
GROUP_SIZES = [2, 4, 8] + [16] * 8 + [10, 4, 1]  # staggered spin-up/down
assert sum(GROUP_SIZES) == KCHUNKS


def split_multiwaits(nc):
    """nix-walrus accepts at most ONE sync-wait per instruction; Tile attaches
    many. Hoist all but the last wait onto single-wait NoOps inserted just
    before the instruction, on the same engine."""
    n_split = 0
    for f in nc.m.functions:
        for bb in f.blocks:
            il = list(bb.instructions)
            new = []
            changed = False
            for ins in il:
                si = ins.sync_info
                if si is not None and si.on_wait is not None and len(si.on_wait) > 1:
                    waits = list(si.on_wait)
                    for k, w in enumerate(waits[:-1]):
                        nop = mybir.InstNoOp(
                            name=f"{ins.name}-wsplit{k}", ins=[], outs=[]
                        )
                        nop.engine = ins.engine
                        nop.sync_info = mybir.SyncInfo(on_wait=[w], on_update=[])
                        new.append(nop)
                    ins.sync_info = mybir.SyncInfo(
                        on_wait=waits[-1:], on_update=list(si.on_update or [])
                    )
                    changed = True
                    n_split += 1
                new.append(ins)
            if changed:
                bb.instructions = new
    return n_split


def build_bass(upc: int, nbpc: int):
    """upc: unique users per core; nbpc: batch entries per core."""
    f32 = mybir.dt.float32
    bf16 = mybir.dt.bfloat16
    f16 = mybir.dt.float16
    i32 = mybir.dt.int32
    EQ = mybir.AluOpType.is_equal
    MUL = mybir.AluOpType.mult

    assert nbpc % 128 == 0
    jg = nbpc // 128

    nc = bass.Bass(trn_type="TRN2")

    # ---- I/O ----
    # lt is partition-major: lt[p, c, u] = label.T[c*128 + p, u]
    LT = nc.dram_tensor("lt", [128, KCHUNKS, upc], f32, kind="ExternalInput")
    T_pm = nc.dram_tensor("t_pm", [128, KCHUNKS * MAUG], bf16, kind="ExternalInput")
    T_cl = nc.dram_tensor("t_cl", [128, MAUG], bf16, kind="ExternalInput")
    EYE = nc.dram_tensor("eye66", [MAUG, MAUG], f32, kind="ExternalInput")
    IOTA = nc.dram_tensor("iota256", [128, CLUSTER], f32, kind="ExternalInput")
    CID = nc.dram_tensor("cid_pm", [128, KCHUNKS], f32, kind="ExternalInput")
    PCOL = nc.dram_tensor("pcol", [128, 2], f32, kind="ExternalInput")
    CPR = nc.dram_tensor("cpr", [128, nbpc], f32, kind="ExternalInput")
    CNR = nc.dram_tensor("cnr", [128, nbpc], f32, kind="ExternalInput")
    UT = nc.dram_tensor("user_table", [NUM_USERS, DIM], f32, kind="ExternalInput")
    IT = nc.dram_tensor("item_table", [NUM_ITEMS + 1, DIM], f32, kind="ExternalInput")
    IDX = {}
    for nm in ("uidx", "pidx", "nidx"):
        IDX[nm] = nc.dram_tensor(nm, [128, jg], i32, kind="ExternalInput")

    UNI = nc.dram_tensor("uni_part", [upc, DIM], f32, kind="ExternalOutput")
    EMB = {}
    for nm in ("ue_out", "pe_out", "ne_out"):
        EMB[nm] = nc.dram_tensor(nm, [nbpc, DIM], f32, kind="ExternalOutput")
    PCT = nc.dram_tensor("pct_out", [DIM, nbpc], f32, kind="ExternalOutput")
    NCT = nc.dram_tensor("nct_out", [DIM, nbpc], f32, kind="ExternalOutput")

    blocks = []
    o = 0
    while o < upc:
        blocks.append((o, min(128, upc - o)))
        o += 128
    nA = min(512, upc)
    gmax = max(GROUP_SIZES)

    with tile.TileContext(nc) as tc:
        with (
            tc.tile_pool(name="const", bufs=1) as cpool,
            tc.tile_pool(name="ltp", bufs=2) as ltpool,
            tc.tile_pool(name="ohp", bufs=2) as ohpool,
            tc.tile_pool(name="acc", bufs=1, space="PSUM") as accpool,
            tc.tile_pool(name="tp", bufs=2, space="PSUM") as tppool,
            tc.tile_pool(name="outp", bufs=3) as outpool,
        ):
            # ---- constants into SBUF ----
            # Two HWDGE rings: T_aug pieces on the sync ring (piecewise, so
            # early chunks' weights arrive with the early lt groups); every
            # small constant on the scalar ring so nothing queues behind the
            # 2.6MB T_aug stream.
            t_sb = cpool.tile([128, KCHUNKS, MAUG], bf16)
            t_view = T_pm[:].rearrange("p (c m) -> p c m", m=MAUG)
            tb = 0
            for piece in (GROUP_SIZES[0], GROUP_SIZES[1], GROUP_SIZES[2],
                          16, KCHUNKS):
                te = min(tb + piece, KCHUNKS)
                nc.sync.dma_start(t_sb[:, tb:te, :], t_view[:, tb:te, :])
                tb = te
                if tb == KCHUNKS:
                    break
            iota_sb = cpool.tile([128, CLUSTER], f32)
            nc.scalar.dma_start(iota_sb[:], IOTA[:])
            cid_sb = cpool.tile([128, KCHUNKS], f32)
            nc.scalar.dma_start(cid_sb[:], CID[:])
            tcl_sb = cpool.tile([128, MAUG], bf16)
            nc.scalar.dma_start(tcl_sb[:], T_cl[:])
            pcol_sb = cpool.tile([128, 2], f32)
            nc.scalar.dma_start(pcol_sb[:], PCOL[:])
            idx_sb = {}
            g_sb = {}
            for nm, h in IDX.items():
                s = cpool.tile([128, jg], i32, name=f"idx_{nm}")
                nc.scalar.dma_start(s[:], h[:])
                idx_sb[nm] = s
                g_sb[nm] = cpool.tile([128, jg, DIM], f32, name=f"g_{nm}")
            eye_sb = cpool.tile([MAUG, MAUG], f32)
            nc.scalar.dma_start(eye_sb[:], EYE[:])
            cpr_sb = cpool.tile([128, nbpc], f32)
            nc.scalar.dma_start(cpr_sb[:], CPR[:])
            cnr_sb = cpool.tile([128, nbpc], f32)
            nc.scalar.dma_start(cnr_sb[:], CNR[:])

            # one [128]-row slice of an embedding gather
            gsrc = {"uidx": UT[:], "pidx": IT[:], "nidx": IT[:]}

            def gather_slice(nm, j):
                nc.gpsimd.indirect_dma_start(
                    out=g_sb[nm][:, j, :],
                    out_offset=None,
                    in_=gsrc[nm],
                    in_offset=bass.IndirectOffsetOnAxis(
                        ap=idx_sb[nm][:, j : j + 1], axis=0
                    ),
                )

            gather_slices = [(nm, j) for nm in ("uidx", "pidx", "nidx")
                             for j in range(jg)]

            # ---- psum accumulators ----
            accA = accpool.tile([MAUG, nA], f32)
            accB = (
                accpool.tile([MAUG, upc - nA], f32, name="accB")
                if upc > nA
                else None
            )
            accC = accpool.tile([MAUG, CLUSTER], f32)

            lt_view = LT[:]

            # ---- main streaming loop ----
            c0 = 0
            n_groups = len(GROUP_SIZES)
            for g, gs in enumerate(GROUP_SIZES):
                lt = ltpool.tile([128, gmax, upc], bf16, name="lt_tile")
                nc.gpsimd.dma_start(lt[:, 0:gs, :], lt_view[:, c0 : c0 + gs, :])
                # spread the embedding-row gathers through the stream so
                # their descriptor generation hides under the big loads
                if g >= 1:
                    k0 = (g - 1) * len(gather_slices) // (n_groups - 1)
                    k1 = g * len(gather_slices) // (n_groups - 1)
                    for nm, j in gather_slices[k0:k1]:
                        gather_slice(nm, j)
                for j in range(gs):
                    c = c0 + j
                    st = c == 0
                    sp = c == KCHUNKS - 1
                    lhs = t_sb[:, c, :]
                    nc.tensor.matmul(
                        accA[:], lhs, lt[:, j, 0:nA], start=st, stop=sp
                    )
                    if accB is not None:
                        nc.tensor.matmul(
                            accB[:], lhs, lt[:, j, nA:upc], start=st, stop=sp
                        )
                    oh = ohpool.tile([128, CLUSTER], bf16, name="oh")
                    nc.vector.tensor_scalar(
                        oh[:], iota_sb[:], cid_sb[:, c : c + 1], None, EQ
                    )
                    nc.tensor.matmul(
                        accC[:],
                        tcl_sb[:] if sp else lhs,
                        oh[:],
                        start=st,
                        stop=sp,
                    )
                c0 += gs

            # ---- embedding gather writeback ----
            for nm, out in (("uidx", EMB["ue_out"]), ("pidx", EMB["pe_out"]),
                            ("nidx", EMB["ne_out"])):
                nc.scalar.dma_start(
                    out[:].rearrange("(j p) d -> p j d", p=128), g_sb[nm][:]
                )

            # ---- finalize centers (emitted first: longest tail chain) ----
            c_sb = outpool.tile([MAUG, CLUSTER], f32, bufs=1)
            nc.vector.tensor_copy(c_sb[:], accC[:])
            nc.vector.tensor_scalar(
                c_sb[64:65, :], c_sb[64:65, :], 1.0, None, mybir.AluOpType.max
            )
            ce = []
            for h in range(2):
                ctp = tppool.tile([128, MAUG], f32, name=f"ctp{h}", tag="tps")
                nc.tensor.matmul(
                    ctp[:], c_sb[:, h * 128 : (h + 1) * 128], eye_sb[:],
                    is_transpose=True,
                )
                rc = outpool.tile([128, 1], f32, name=f"rc{h}")
                nc.vector.reciprocal(rc[:], ctp[:, 64:65])
                ce_sb = outpool.tile([128, DIM], f16, name=f"ce_sb{h}", bufs=1)
                nc.vector.tensor_scalar(ce_sb[:], ctp[:, 0:DIM], rc[:], None, MUL)
                ce.append(ce_sb)

            # ---- pos/neg centers via onehot matmuls ----
            # onehot[p, b] = (cid[batch b] == h*128 + p); each column has
            # exactly one nonzero, so centers.T @ onehot is a gather (fp16
            # rounds the gathered center values only).
            for rep_sb, out in ((cpr_sb, PCT), (cnr_sb, NCT)):
                pcps = tppool.tile([DIM, nbpc], f32, name="pcps", tag="pcps",
                                   bufs=1)
                for h in range(2):
                    oh2 = ohpool.tile([128, nbpc], f16, name="oh2")
                    nc.vector.tensor_scalar(
                        oh2[:], rep_sb[:], pcol_sb[:, h : h + 1], None, EQ
                    )
                    for q in range(0, nbpc, 512):
                        qe = min(q + 512, nbpc)
                        nc.tensor.matmul(
                            pcps[:, q:qe], ce[h][:], oh2[:, q:qe],
                            start=(h == 0), stop=(h == 1),
                        )
                pct_sb = outpool.tile([DIM, nbpc], f32, name="pct_sb")
                nc.vector.tensor_copy(pct_sb[:], pcps[:])
                nc.scalar.dma_start(out[:], pct_sb[:])

            # ---- finalize uni_center ----
            p_sb = outpool.tile([MAUG, upc], f32, bufs=1)
            nc.vector.tensor_copy(p_sb[:, 0:nA], accA[:])
            if accB is not None:
                nc.vector.tensor_copy(p_sb[:, nA:upc], accB[:])
            for o, blk in blocks:
                ptp = tppool.tile([128, MAUG], f32, name="ptp", tag="tps")
                nc.tensor.matmul(
                    ptp[0:blk, :], p_sb[:, o : o + blk], eye_sb[:],
                    is_transpose=True,
                )
                r = outpool.tile([128, 1], f32, name="recip")
                nc.vector.reciprocal(r[0:blk, :], ptp[0:blk, 64:65])
                u_sb = outpool.tile([128, DIM], f32, name="u_sb")
                nc.vector.tensor_scalar(
                    u_sb[0:blk, :], ptp[0:blk, 0:DIM], r[0:blk, :], None, MUL
                )
                nc.sync.dma_start(UNI[o : o + blk, :], u_sb[0:blk, :])

    split_multiwaits(nc)
    return nc


# ------------------------- host side -------------------------

def _wrap_idx(idx: np.ndarray) -> np.ndarray:
    """indirect gather layout: element [p, j] = idx[j*128 + p]."""
    n = idx.shape[0]
    return np.ascontiguousarray(idx.astype(np.int32).reshape(n // 128, 128).T)


def host_prep(user, pos, neg, cluster_ids, user_table, item_table, train_label):
    user = np.asarray(user).astype(np.int64)
    pos = np.asarray(pos).astype(np.int64)
    neg = np.asarray(neg).astype(np.int64)
    cluster_ids = np.asarray(cluster_ids).astype(np.int64)
    user_table = np.ascontiguousarray(np.asarray(user_table, dtype=np.float32))
    item_table = np.ascontiguousarray(np.asarray(item_table, dtype=np.float32))
    train_label = np.asarray(train_label, dtype=np.float32)

    uniq, inverse = np.unique(user, return_inverse=True)
    nu = len(uniq)
    upc = -(-nu // (NCORES * 16)) * 16  # per-core users, mult of 16
    upad = upc * NCORES
    uu = np.concatenate([uniq, np.full(upad - nu, uniq[0], dtype=uniq.dtype)])

    # T_aug partition-major [128, KCHUNKS, MAUG] bf16
    t_aug = np.zeros((KPAD, MAUG), np.float32)
    t_aug[: NUM_ITEMS, :DIM] = item_table[:NUM_ITEMS]  # row 20000 zeroed
    t_aug[: NUM_ITEMS + 1, DIM] = 1.0
    t_pm = np.ascontiguousarray(
        t_aug.reshape(KCHUNKS, 128, MAUG).transpose(1, 0, 2).reshape(128, -1)
    ).astype(ml_dtypes.bfloat16)
    # centers variant of the last chunk: real row 20000
    last = np.zeros((128, MAUG), np.float32)
    lo = (KCHUNKS - 1) * 128
    nreal = NUM_ITEMS + 1 - lo
    last[:nreal, :DIM] = item_table[lo : NUM_ITEMS + 1]
    last[:nreal, DIM] = 1.0
    t_cl = last.astype(ml_dtypes.bfloat16)

    eye66 = np.eye(MAUG, dtype=np.float32)
    iota256 = np.broadcast_to(
        np.arange(CLUSTER, dtype=np.float32), (128, CLUSTER)
    ).copy()
    cid_pm = np.full((KPAD,), -1.0, np.float32)
    cid_pm[: NUM_ITEMS + 1] = cluster_ids.astype(np.float32)
    cid_pm = np.ascontiguousarray(cid_pm.reshape(KCHUNKS, 128).T)
    pcol = (np.arange(128, dtype=np.float32)[:, None]
            + np.array([0.0, 128.0], np.float32)[None, :])
    pcol = np.ascontiguousarray(pcol)

    cpos = cluster_ids[pos].astype(np.float32)
    cneg = cluster_ids[neg].astype(np.float32)

    nbpc = BATCH // NCORES
    shared = {
        "t_pm": t_pm,
        "t_cl": t_cl,
        "eye66": eye66,
        "iota256": iota256,
        "cid_pm": cid_pm,
        "pcol": pcol,
        "user_table": user_table,
        "item_table": item_table,
    }
    in_maps = []
    for c in range(NCORES):
        rows = uu[c * upc : (c + 1) * upc]
        gathered = train_label[rows]  # [upc, 20001]
        lt = np.zeros((KPAD, upc), np.float32)
        lt[: NUM_ITEMS + 1, :] = gathered.T
        # partition-major: ltpm[p, c, u] = lt[c*128 + p, u]
        ltpm = np.ascontiguousarray(
            lt.reshape(KCHUNKS, 128, upc).transpose(1, 0, 2)
        )
        bs = slice(c * nbpc, (c + 1) * nbpc)
        m = dict(shared)
        m["lt"] = ltpm
        m["uidx"] = _wrap_idx(user[bs])
        m["pidx"] = _wrap_idx(pos[bs])
        m["nidx"] = _wrap_idx(neg[bs])
        m["cpr"] = np.ascontiguousarray(
            np.broadcast_to(cpos[bs][None, :], (128, nbpc))
        )
        m["cnr"] = np.ascontiguousarray(
            np.broadcast_to(cneg[bs][None, :], (128, nbpc))
        )
        in_maps.append(m)

    meta = {"upc": upc, "nbpc": nbpc, "nu": nu, "inverse": inverse}
    return in_maps, meta


def assemble(results, meta):
    inverse = meta["inverse"]
    uni_unique = np.concatenate([r["uni_part"] for r in results], axis=0)
    uni = uni_unique[inverse]
    ue = np.concatenate([r["ue_out"] for r in results], axis=0)
    pe = np.concatenate([r["pe_out"] for r in results], axis=0)
    ne = np.concatenate([r["ne_out"] for r in results], axis=0)
    pc = np.concatenate([r["pct_out"].T for r in results], axis=0)
    ncen = np.concatenate([r["nct_out"].T for r in results], axis=0)
    return ue, pe, ne, pc, ncen, uni


_CACHE = {}


def _run(in_maps, meta, trace=False):
    from concourse.bass_utils import run_bass_kernel_spmd

    key = (meta["upc"], meta["nbpc"])
    if key not in _CACHE:
        _CACHE[key] = build_bass(*key)
    nc = _CACHE[key]
    res = run_bass_kernel_spmd(
        nc, in_maps, core_ids=list(range(NCORES)), trace=trace
    )
    return res


def kernel(user, pos, neg, cluster_ids, user_table, item_table, train_label):
    """Full (unsharded) inputs -> full outputs, computed on 8 NeuronCores."""
    in_maps, meta = host_prep(
        user, pos, neg, cluster_ids, user_table, item_table, train_label
    )
    res = _run(in_maps, meta)
    return assemble(res.results, meta)
